# revision 21
# baseline (speedup 1.0000x reference)
"""TRN2 Bass kernel for the 2-layer LSTM RL net (nn_Net_32882269618241).

Strategy: the per-(t,b) `done` flags reset the LSTM state constantly, so the
T=512 scan is sharded into 16 time-chunks (2 per core) that each replay a
short warmup window from zero state; every batch element hits a reset inside
the window, so chunk results are exact.  The two chunks on a core run as two
concurrent PE column-tiles (batch-64 stationary each).  Layer 1 and layer 2
run as two passes; layer-1 hidden states round-trip through DRAM transposed
so pass 2 consumes them exactly like pass 1 consumes the input stream.

All matmuls run with float32r operands (full PE rate, ~1e-4 accuracy);
everything else is fp32.  Raw bass (explicit engine programs + semaphores):
the Tile layer emits multi-wait instructions this walrus rejects.
"""

import sys

for _p in ("/opt/trn_rl_repo",):
    if _p not in sys.path:
        sys.path.append(_p)

import numpy as np

import concourse.bass as bass
import concourse.mybir as mybir
from concourse.bass_utils import run_bass_kernel_spmd
from concourse.masks import make_identity

F32 = mybir.dt.float32
F32R = mybir.dt.float32r
U32 = mybir.dt.uint32
AF = mybir.ActivationFunctionType
ALU = mybir.AluOpType

B = 64            # batch
H = 577           # hidden/input size
A = 64            # actions
G = 4 * H         # 2308 gate width
KP = 640          # padded contraction dim (578 used: H + ones row)
NK = KP // 128    # 5 K-chunks
NCORES = 8
# gate psum free-dim chunk boundaries (bank-aligned at 512 f32)
NCH = [(0, 512), (512, 512), (1024, 512), (1536, 512), (2048, G - 2048)]
# index of the last ACT op (1-based si,sf,tg,so) reading each psum bank
LASTREADER = [1, 2, 3, 4, 4]
# ACT incs per tick: si sf tg so tc copies*5
ACTI = 10
PEI = 6           # PE incs per tick: 5 N-chunk groups + transposes
DVEI = 9          # DVE incs per tick: stt, p1, c, h, masks*5


def build(nticks, own, r_rows):
    """Emit the full raw-bass program.

    nticks: ticks per pass (= warmup L + own)
    own:    own steps per chunk (32 for the real problem)
    r_rows: output rows per core (= 2*own*64)
    """
    NT2 = 2 * nticks
    nc = bass.Bass("TRN2", target_bir_lowering=False)

    # ---- DRAM I/O -------------------------------------------------------
    xt_in = nc.dram_tensor("xt_in", [nticks, 128, NK, 128], F32R, kind="ExternalInput")
    wih_in = nc.dram_tensor("wih_in", [2, 128, NK, G], F32R, kind="ExternalInput")
    whh_in = nc.dram_tensor("whh_in", [2, 128, NK, G], F32R, kind="ExternalInput")
    mc_in = nc.dram_tensor("mc_in", [128, nticks], F32, kind="ExternalInput")
    mb_in = nc.dram_tensor("mb_in", [nticks, 128, 128], F32, kind="ExternalInput")
    hw_in = nc.dram_tensor("hw_in", [128, NK, 66], F32R, kind="ExternalInput")

    h0t_dram = nc.dram_tensor("h0t_scratch", [nticks, 128, NK, 128], F32R)
    h1t_dram = nc.dram_tensor("h1t_scratch", [128, NK, r_rows], F32R)

    logits_out = nc.dram_tensor("logits_out", [r_rows, A], F32, kind="ExternalOutput")
    basel_out = nc.dram_tensor("basel_out", [r_rows, 1], F32, kind="ExternalOutput")
    action_out = nc.dram_tensor("action_out", [r_rows, 1], U32, kind="ExternalOutput")
    hT_out = nc.dram_tensor("hT_out", [2, 64, H], F32, kind="ExternalOutput")
    cT_out = nc.dram_tensor("cT_out", [2, 64, H], F32, kind="ExternalOutput")

    NROW = r_rows // 128          # head row-chunks
    NROWH = NROW // 2             # per half
    RH = r_rows // 2              # columns per h1t half

    # --- store-DMA bookkeeping: per-parity cumulative totals -------------
    st_after_tick = []               # value of s_st[g%2] after tick g's stores
    st_tot = [0, 0]
    for g in range(NT2):
        p, t = divmod(g, nticks)
        add = 0
        if p == 0:
            add += 16                      # h0T write
        elif t >= nticks - own:
            add += 32                      # two h1T writes
        if t == nticks - 1:
            add += 32                      # final h/c states
        st_tot[g % 2] += add
        st_after_tick.append(st_tot[g % 2])

    from contextlib import ExitStack

    es = ExitStack()
    with es:
        block = es.enter_context(nc.Block())
        sem = lambda n: es.enter_context(nc.semaphore(n))
        _nm = [0]

        def sb(shape, dt=F32):
            _nm[0] += 1
            return es.enter_context(nc.sbuf_tensor(f"sb{_nm[0]}", shape, dt))

        def ps(shape, dt=F32):
            _nm[0] += 1
            return es.enter_context(nc.psum_tensor(f"ps{_nm[0]}", shape, dt))
        s_x = [sem("s_x0"), sem("s_x1"), sem("s_x2")]
        s_mb = [sem("s_mb0"), sem("s_mb1")]
        s_st = [sem("s_ge"), sem("s_go")]
        s_w1, s_w2, s_wh, s_h1 = sem("s_w1"), sem("s_w2"), sem("s_wh"), sem("s_h1")
        s_pe, s_act, s_dve = sem("s_pe"), sem("s_act"), sem("s_dve")
        s_hd = [sem("s_hde"), sem("s_hdo")]
        s_i = sem("s_i")
        s_mc = sem("s_mc")
        w_ih = sb([128, NK, G], F32R)
        w_hh = sb([128, NK, G], F32R)
        xl0, xl1, xl2 = (
            sb([128, NK, 128], F32R),
            sb([128, NK, 128], F32R),
            sb([128, NK, 128], F32R),
        )
        hm0, hm1 = sb([128, NK, 128], F32R), sb([128, NK, 128], F32R)
        hu0, hu1 = sb([128, NK, 128], F32R), sb([128, NK, 128], F32R)
        c_sb = sb([128, H])
        tc_sb = sb([128, H])
        hnew_sb = sb([128, KP])
        si_sb, sf_sb, tg_sb, so_sb, tmp_sb = (
            sb([128, H]), sb([128, H]), sb([128, H]), sb([128, H]), sb([128, H])
        )
        mc_sb = sb([128, nticks])
        mb0_sb, mb1_sb = sb([128, 128]), sb([128, 128])
        ident = sb([128, 128])
        h1t_sb = sb([128, NK, RH], F32R)
        hw_sb = sb([128, NK, 66], F32R)
        lg0_sb, lg1_sb = sb([128, 66]), sb([128, 66])
        mx0_sb, mx1_sb = sb([128, 8]), sb([128, 8])
        mi0_sb, mi1_sb = sb([128, 8], U32), sb([128, 8], U32)
        gates_ps = ps([128, 2560])  # bank-aligned partition stride
        tp_ps = ps([128, NK, 128])

        xl = [xl0, xl1, xl2]
        hm = [hm0, hm1]
        hu = [hu0, hu1]
        mb = [mb0_sb, mb1_sb]
        lg = [lg0_sb, lg1_sb]
        mx = [mx0_sb, mx1_sb]
        mi = [mi0_sb, mi1_sb]

        def r32(ap):
            return ap.bitcast(F32R)

        # ---------------- SYNC: x-stream loads + weight swaps ------------
        @block.sync
        def _(sy):
            sy.dma_start(out=mc_sb[:], in_=mc_in[:]).then_inc(s_mc, 16)
            for g in range(NT2):
                p, t = divmod(g, nticks)
                if g == nticks + 2:
                    # pass-2 weights, before any load that needs pass-2 PE
                    # progress (W region free once pass-1 PE work is done)
                    sy.wait_ge(s_pe, PEI * nticks)
                    sy.dma_start(out=w_ih[:], in_=wih_in[1]).then_inc(s_w2, 16)
                    sy.dma_start(out=w_hh[:], in_=whh_in[1]).then_inc(s_w2, 16)
                if g >= 3:
                    sy.wait_ge(s_pe, PEI * (g - 3) + PEI)
                src = xt_in[t] if p == 0 else h0t_dram[t]
                sy.dma_start(out=xl[g % 3][:], in_=src).then_inc(s_x[g % 3], 16)
            if NT2 < nticks + 2:
                sy.wait_ge(s_pe, PEI * nticks)
                sy.dma_start(out=w_ih[:], in_=wih_in[1]).then_inc(s_w2, 16)
                sy.dma_start(out=w_hh[:], in_=whh_in[1]).then_inc(s_w2, 16)
            # heads weights + h1t halves
            sy.dma_start(out=hw_sb[:], in_=hw_in[:]).then_inc(s_wh, 16)
            sy.wait_ge(s_st[0], st_tot[0])      # all h1T writes complete
            sy.wait_ge(s_st[1], st_tot[1])
            sy.dma_start(out=h1t_sb[:], in_=h1t_dram[:, :, 0:RH]).then_inc(s_h1, 16)
            sy.wait_ge(s_pe, PEI * NT2 + NROWH)
            sy.dma_start(out=h1t_sb[:], in_=h1t_dram[:, :, RH:]).then_inc(s_h1, 16)
            for q in range(3):
                sy.wait_ge(s_x[q], 16 * len(range(q, NT2, 3)))
            sy.wait_ge(s_mc, 16)

        # ---------------- PE: matmuls + transposes -----------------------
        @block.tensor
        def _(pe):
            for g in range(NT2):
                p, t = divmod(g, nticks)
                if g == 0:
                    pe.wait_ge(s_i, 8)
                    pe.wait_ge(s_w1, 32)
                if p == 1 and t == 0:
                    pe.wait_ge(s_w2, 32)
                if g >= 1:
                    pe.wait_ge(s_dve, DVEI * (g - 1) + DVEI)
                pe.wait_ge(s_x[g % 3], 16 * (g // 3 + 1))
                xt = xl[g % 3]
                hmt = hm[(g - 1) % 2]
                for ci, (n0, w) in enumerate(NCH):
                    if g >= 1:
                        pe.wait_ge(s_act, ACTI * (g - 1) + LASTREADER[ci])
                    mm = None
                    for kk in range(2 * NK):
                        ih = kk < NK
                        kc = kk if ih else kk - NK
                        lt = xt if ih else hmt
                        mm = pe.matmul(
                            gates_ps[:, n0 : n0 + w],
                            lt[:, kc, :],
                            (w_ih if ih else w_hh)[:, kc, n0 : n0 + w],
                            start=(kk == 0),
                            stop=(kk == 2 * NK - 1),
                            skip_group_check=True,
                        )
                    mm.then_inc(s_pe, 1)
                # transposes of h_new -> tp_ps
                pe.wait_ge(s_dve, DVEI * g + 4)
                if g >= 1:
                    pe.wait_ge(s_act, ACTI * (g - 1) + ACTI)
                tp = None
                for kc in range(NK):
                    tp = pe.transpose(
                        tp_ps[:, kc, :], hnew_sb[:, 128 * kc : 128 * kc + 128], ident[:]
                    )
                tp.then_inc(s_pe, 1)

            # ------- heads matmuls -------
            for half in range(2):
                for rr in range(NROWH):
                    r = half * NROWH + rr
                    if rr == 0:
                        pe.wait_ge(s_wh, 16)
                        pe.wait_ge(s_h1, 16 * (half + 1))
                    if r < 2:
                        pe.wait_ge(s_act, ACTI * NT2)
                    else:
                        pe.wait_ge(s_act, ACTI * NT2 + (r - 2) + 1)
                    mm = None
                    for kc in range(NK):
                        mm = pe.matmul(
                            tp_ps[:, 4 * (r % 2), 0:66],
                            r32(h1t_sb[:, kc, 128 * rr : 128 * rr + 128]),
                            r32(hw_sb[:, kc, :]),
                            start=(kc == 0),
                            stop=(kc == NK - 1),
                            skip_group_check=True,
                        )
                    mm.then_inc(s_pe, 1)

        # ---------------- ACT: activations + psum copies -----------------
        @block.scalar
        def _(ac):
            for g in range(NT2):
                base = PEI * g
                ac.wait_ge(s_pe, base + 2)
                ac.activation(si_sb[:], gates_ps[:, 0:H], AF.Sigmoid).then_inc(s_act, 1)
                ac.wait_ge(s_pe, base + 3)
                ac.activation(sf_sb[:], gates_ps[:, H : 2 * H], AF.Sigmoid).then_inc(
                    s_act, 1
                )
                ac.wait_ge(s_pe, base + 4)
                ac.activation(tg_sb[:], gates_ps[:, 2 * H : 3 * H], AF.Tanh).then_inc(
                    s_act, 1
                )
                ac.wait_ge(s_pe, base + 5)
                ac.activation(so_sb[:], gates_ps[:, 3 * H : 4 * H], AF.Sigmoid).then_inc(
                    s_act, 1
                )
                ac.wait_ge(s_dve, DVEI * g + 3)
                ac.activation(tc_sb[:], c_sb[:], AF.Tanh).then_inc(s_act, 1)
                ac.wait_ge(s_pe, base + 6)
                if g >= 2:
                    # hu[g%2] readers from tick g-2 (h0T/h1T DMAs) must be done
                    ac.wait_ge(s_st[g % 2], st_after_tick[g - 2])
                hut = hu[g % 2]
                for kc in range(NK):
                    ac.activation(hut[:, kc, :], tp_ps[:, kc, :], AF.Copy).then_inc(
                        s_act, 1
                    )

            # ------- heads psum->sbuf copies -------
            for r in range(NROW):
                ac.wait_ge(s_pe, PEI * NT2 + r + 1)
                if r >= 2:
                    ac.wait_ge(s_hd[r % 2], 48 * ((r - 2) // 2 + 1))
                ac.activation(
                    lg[r % 2][:], tp_ps[:, 4 * (r % 2), 0:66], AF.Copy
                ).then_inc(s_act, 1)

        # ---------------- DVE: cell state math + masking -----------------
        @block.vector
        def _(dv):
            dv.wait_ge(s_i, 8)
            dv.wait_ge(s_mc, 16)
            for g in range(NT2):
                p, t = divmod(g, nticks)
                abase = ACTI * g
                if p == 1 and t == 0:
                    # final-state DMAs of pass 1 read c/hnew must be done
                    dv.wait_ge(s_st[(nticks - 1) % 2], st_after_tick[nticks - 1])
                dv.wait_ge(s_act, abase + 2)
                dv.scalar_tensor_tensor(
                    tmp_sb[:], c_sb[:], mc_sb[:, t : t + 1], sf_sb[:], ALU.mult, ALU.mult
                ).then_inc(s_dve, 1)
                dv.wait_ge(s_act, abase + 3)
                dv.tensor_tensor(si_sb[:], si_sb[:], tg_sb[:], ALU.mult).then_inc(
                    s_dve, 1
                )
                dv.wait_ge(s_dve, DVEI * g + 2)   # own stt+p1 drained
                dv.tensor_tensor(c_sb[:], tmp_sb[:], si_sb[:], ALU.add).then_inc(
                    s_dve, 1
                )
                dv.wait_ge(s_act, abase + 5)
                dv.tensor_tensor(
                    hnew_sb[:, 0:H], so_sb[:], tc_sb[:], ALU.mult
                ).then_inc(s_dve, 1)
                dv.wait_ge(s_act, abase + ACTI)
                dv.wait_ge(s_mb[g % 2], 16 * (g // 2 + 1))
                hut, hmt = hu[g % 2], hm[g % 2]
                mbt = mb[g % 2]
                for kc in range(NK):
                    dv.tensor_tensor(
                        hmt[:, kc, :], hut[:, kc, :], mbt[:], ALU.mult
                    ).then_inc(s_dve, 1)

            # ------- heads argmax -------
            for r in range(NROW):
                dv.wait_ge(s_act, ACTI * NT2 + r + 1)
                if r >= 2:
                    dv.wait_ge(s_hd[r % 2], 48 * ((r - 2) // 2 + 1))
                dv.max(mx[r % 2][:], lg[r % 2][:, 0:A]).then_inc(s_dve, 1)
                dv.wait_ge(s_dve, DVEI * NT2 + 2 * r + 1)
                dv.max_index(mi[r % 2][:], mx[r % 2][:], lg[r % 2][:, 0:A]).then_inc(
                    s_dve, 1
                )

        # ---------------- GPSIMD: init, mask loads, store DMAs -----------
        @block.gpsimd
        def _(gp):
            gp.memset(hm0[:].bitcast(F32), 0.0).then_inc(s_i, 1)
            gp.memset(hm1[:].bitcast(F32), 0.0).then_inc(s_i, 1)
            gp.memset(c_sb[:], 0.0).then_inc(s_i, 1)
            gp.memset(hnew_sb[:, 0:H], 0.0).then_inc(s_i, 1)
            gp.memset(hnew_sb[:, H : H + 1], 1.0).then_inc(s_i, 1)
            gp.memset(hnew_sb[:, H + 1 :], 0.0).then_inc(s_i, 1)
            gp.memset(ident[:], 0.0).then_inc(s_i, 1)
            gp.wait_ge(s_i, 7)
            gp.affine_select(
                out=ident[:],
                in_=ident[:],
                compare_op=ALU.not_equal,
                fill=1.0,
                base=0,
                pattern=[[-1, 128]],
                channel_multiplier=1,
            ).then_inc(s_i, 1)
            gp.dma_start(out=w_ih[:], in_=wih_in[0]).then_inc(s_w1, 16)
            gp.dma_start(out=w_hh[:], in_=whh_in[0]).then_inc(s_w1, 16)
            gp.dma_start(out=mb0_sb[:], in_=mb_in[0]).then_inc(s_mb[0], 16)

            for g in range(NT2):
                p, t = divmod(g, nticks)
                # next mb mask (t+1 of same pass; mb_in[0] again at boundary)
                if g + 1 < NT2:
                    _, nx_t = divmod(g + 1, nticks)
                    if g >= 1:
                        gp.wait_ge(s_dve, DVEI * (g - 1) + DVEI)
                    gp.dma_start(out=mb[(g + 1) % 2][:], in_=mb_in[nx_t]).then_inc(
                        s_mb[(g + 1) % 2], 16
                    )
                gp.wait_ge(s_act, ACTI * g + ACTI)
                hut = hu[g % 2]
                sgp = s_st[g % 2]
                if p == 0:
                    gp.dma_start(out=h0t_dram[t], in_=hut[:]).then_inc(sgp, 16)
                elif t >= nticks - own:
                    ro = (t - (nticks - own)) * 64
                    gp.dma_start(
                        out=h1t_dram[:, :, ro : ro + 64], in_=hut[:, :, 0:64]
                    ).then_inc(sgp, 16)
                    gp.dma_start(
                        out=h1t_dram[:, :, own * 64 + ro : own * 64 + ro + 64],
                        in_=hut[:, :, 64:128],
                    ).then_inc(sgp, 16)
                if t == nticks - 1:
                    gp.wait_ge(s_dve, DVEI * g + 4)
                    gp.dma_start(out=hT_out[p], in_=hnew_sb[64:128, 0:H]).then_inc(
                        sgp, 16
                    )
                    gp.dma_start(out=cT_out[p], in_=c_sb[64:128, 0:H]).then_inc(
                        sgp, 16
                    )
            gp.wait_ge(s_st[0], st_tot[0])
            gp.wait_ge(s_st[1], st_tot[1])

        # ---------------- heads output DMAs (gpsimd, appended) ------------
        @block.gpsimd
        def _(gp):
            for r in range(NROW):
                r0 = 128 * r
                gp.wait_ge(s_dve, DVEI * NT2 + 2 * (r + 1))
                lgt = lg[r % 2]
                gp.dma_start(out=logits_out[r0 : r0 + 128, :], in_=lgt[:, 0:A]).then_inc(
                    s_hd[r % 2], 16
                )
                gp.dma_start(
                    out=basel_out[r0 : r0 + 128, :], in_=lgt[:, A : A + 1]
                ).then_inc(s_hd[r % 2], 16)
                gp.dma_start(
                    out=action_out[r0 : r0 + 128, :], in_=mi[r % 2][:, 0:1]
                ).then_inc(s_hd[r % 2], 16)
            gp.wait_ge(s_hd[0], 48 * ((NROW + 1) // 2))
            gp.wait_ge(s_hd[1], 48 * (NROW // 2))

    return nc


# ======================= host-side preparation ==========================


def compute_warmup(done, own, nchunks):
    """Smallest L such that every chunk boundary has a reset within L steps
    for every batch element."""
    L = 1
    for c in range(1, nchunks):
        t0 = own * c
        for b in range(done.shape[1]):
            idx = np.nonzero(done[:t0, b])[0]
            need = t0 if len(idx) == 0 else t0 - idx[-1]
            L = max(L, int(need))
    return L


def host_prep(inputs, own, L):
    """Build per-core input arrays. Returns (in_maps, nticks)."""
    frame = np.asarray(inputs["frame"], np.float32)
    last_action = np.asarray(inputs["last_action"])
    reward = np.asarray(inputs["reward"], np.float32)
    done = np.asarray(inputs["done"])
    T = frame.shape[0]
    nticks = own + L

    # core_in [T, B, 578]: frame | clip(reward) | one_hot | ones
    core_in = np.zeros((T, B, H + 1), np.float32)
    nf = frame.shape[2]
    core_in[:, :, :nf] = frame
    core_in[:, :, nf] = np.clip(reward, -1.0, 1.0)
    tt, bb = np.meshgrid(np.arange(T), np.arange(B), indexing="ij")
    core_in[tt, bb, nf + 1 + last_action] = 1.0
    core_in[:, :, H] = 1.0

    notdone = (~done).astype(np.float32)

    def wT(w, b1, b2):
        out = np.zeros((KP, G), np.float32)
        out[:H] = np.asarray(w, np.float32).T
        out[H] = np.asarray(b1, np.float32) + np.asarray(b2, np.float32)
        return out

    wih = np.stack(
        [
            wT(inputs["w_ih0"], inputs["b_ih0"], inputs["b_hh0"]),
            wT(inputs["w_ih1"], inputs["b_ih1"], inputs["b_hh1"]),
        ]
    )
    whh = np.zeros((2, KP, G), np.float32)
    whh[0, :H] = np.asarray(inputs["w_hh0"], np.float32).T
    whh[1, :H] = np.asarray(inputs["w_hh1"], np.float32).T

    hw = np.zeros((KP, 66), np.float32)
    hw[:H, :A] = np.asarray(inputs["policy_w"], np.float32).T
    hw[:H, A] = np.asarray(inputs["baseline_w"], np.float32)[0]
    hw[H, :A] = np.asarray(inputs["policy_b"], np.float32)
    hw[H, A] = np.asarray(inputs["baseline_b"], np.float32)[0]

    def chunked(kpg):  # [KP, N] -> [128, NK, N]
        return np.ascontiguousarray(
            kpg.reshape(NK, 128, kpg.shape[1]).transpose(1, 0, 2)
        )

    wih_c = np.stack([chunked(wih[0]), chunked(wih[1])])
    whh_c = np.stack([chunked(whh[0]), chunked(whh[1])])
    hw_c = chunked(hw)

    in_maps = []
    for k in range(NCORES):
        starts = (own * (2 * k) - L, own * (2 * k + 1) - L)
        xt = np.zeros((nticks, 128, NK, 128), np.float32)
        mc = np.zeros((128, nticks), np.float32)
        mbm = np.zeros((nticks, 128, 128), np.float32)
        for tau in range(nticks):
            for j, s0 in enumerate(starts):
                t = s0 + tau
                if t >= 0:
                    pad = np.zeros((KP, 64), np.float32)
                    pad[: H + 1] = core_in[t].T
                    xt[tau, :, :, 64 * j : 64 * j + 64] = pad.reshape(
                        NK, 128, 64
                    ).transpose(1, 0, 2)
                    if tau > 0 and t > 0:
                        mc[64 * j : 64 * j + 64, tau] = notdone[t]
                t1 = s0 + tau + 1
                if tau + 1 < nticks and t1 > 0:
                    mbm[tau, :, 64 * j : 64 * j + 64] = notdone[t1][None, :]
        in_maps.append(
            {
                "xt_in": xt,
                "wih_in": wih_c,
                "whh_in": whh_c,
                "mc_in": mc,
                "mb_in": mbm,
                "hw_in": hw_c,
            }
        )
    return in_maps, nticks


_CACHE = {}


def kernel(_trace=False, **inputs):
    frame = np.asarray(inputs["frame"], np.float32)
    T = frame.shape[0]
    own = T // (2 * NCORES)
    done = np.asarray(inputs["done"])
    assert not np.asarray(inputs["h0"]).any() and not np.asarray(inputs["c0"]).any()

    L = compute_warmup(done, own, 2 * NCORES) + 2
    in_maps, nticks = host_prep(inputs, own, L)
    r_rows = 2 * own * 64

    key = (nticks, own, r_rows)
    if key not in _CACHE:
        _CACHE[key] = build(nticks, own, r_rows)
    nc = _CACHE[key]

    r = run_bass_kernel_spmd(
        nc, in_maps, core_ids=list(range(NCORES)), trace=_trace
    )
    if _trace:
        kernel.last_profile = r
        print("HW exec time:", r.exec_time_ns, "ns")
    res = r.results

    logits = np.zeros((T * B, A), np.float32)
    baseline = np.zeros((T * B,), np.float32)
    action = np.zeros((T * B,), np.int32)
    for k in range(NCORES):
        r0 = r_rows * k
        logits[r0 : r0 + r_rows] = res[k]["logits_out"]
        baseline[r0 : r0 + r_rows] = res[k]["basel_out"][:, 0]
        action[r0 : r0 + r_rows] = res[k]["action_out"][:, 0].astype(np.int32)
    hT = res[NCORES - 1]["hT_out"]
    cT = res[NCORES - 1]["cT_out"]
    return (
        logits.reshape(T, B, A),
        baseline.reshape(T, B),
        action.reshape(T, B),
        hT,
        cT,
    )
